# revision 15
# baseline (speedup 1.0000x reference)
"""Trainium2 Bass kernel for KNN-upsample (MLP on down points + KNN mean + residual).

Contract: kernel(**inputs) takes FULL numpy inputs (as produced by
setup_inputs) and returns the FULL (LU, N, D_OUT) float32 output.

Sharding: batch axis N=8 -> one NeuronCore per batch column (data
parallel, no cross-core communication).

Per-core device program (all bf16 on the wires, fp32 PSUM accumulate):
  Phase A: h2x[m, :] = relu(down[m, :] @ W1 + b1) @ (W2/3)   -> DRAM scratch (bf16)
  Phase B: out[i, :] = (up[i, :] + b2) + sum_k h2x[idx[i, k], :]
           gather via gpsimd.dma_gather (bf16 256B rows), rotated across the
           4 SWDGE queues so descriptor generation for consecutive chunks
           runs on different Q7 core pairs and pipelines; deep tile-pool
           buffering (g=5/ix=6) keeps that pipeline full.
           K-sum + residual on DVE in bf16 (2x mode, contiguous rows).

Host-side preprocessing (pure data-layout work):
  - down column transposed to (D_IN, LD) bf16 so matmul lhsT/rhs layouts are native
  - W2 pre-scaled by 1/3 (folds the k-mean), b2 folded into up; weights bf16
  - idx converted to int32 offset tables offt[c, p, 3u+k] = idx[point(c,p,u), k]
    where point i = c*CHUNK + 128*u + p, so gathered rows land at g[p, 3u+k, :]
  - up / out stored chunk-major [c, p, u, f] bf16 so DMAs are contiguous per
    partition; the inverse permutation (and fp32 cast) applied on host
"""

import numpy as np
import ml_dtypes
from contextlib import ExitStack

import concourse.bacc as bacc
import concourse.bass as bass
import concourse.tile as tile
import concourse.mybir as mybir
from concourse.bass_utils import run_bass_kernel_spmd

LD, LU, N, D_IN, D_OUT, K = 16384, 65536, 8, 256, 128, 3

CHUNK = 4096            # upsample points per gather chunk
MLP_BLOCK = 512         # down points per MLP compute block
NCORES = 8
GATHER_MODE = "dma_gather"   # "dma_gather" | "indirect"

F32 = mybir.dt.float32
BF16 = mybir.dt.bfloat16
I32 = mybir.dt.int32
I16 = mybir.dt.int16
BF = ml_dtypes.bfloat16

_BUILD_CACHE = {}


def _build(ld=LD, lu=LU, d_in=D_IN, d_out=D_OUT, chunk=CHUNK, mlp_block=MLP_BLOCK):
    """Build + compile the per-core Bass program (identical on all cores)."""
    key = (ld, lu, d_in, d_out, chunk, mlp_block)
    if key in _BUILD_CACHE:
        return _BUILD_CACHE[key]

    nchunk = lu // chunk
    upc = chunk // 128                  # 128-point groups per chunk
    nj = K * upc                        # gather slots per partition per chunk
    nkb = d_in // 128                   # K-tiles for matmul 1
    nblk = ld // mlp_block              # MLP compute blocks
    spb = mlp_block // 128              # 128-point sub-tiles per MLP block

    # 4 SWDGE queues: consecutive gathers rotate queues so each has its own
    # descriptor ring + completion semaphore; otherwise gather c+1 stalls on
    # gather c's DMA drain (~16us bubble per chunk).
    nc = bacc.Bacc("TRN2", target_bir_lowering=False, debug=False,
                   num_swdge_queues=4)

    downt_d = nc.dram_tensor("downt", (d_in, ld), BF16, kind="ExternalInput")
    w1_d = nc.dram_tensor("w1", (d_in, d_out), BF16, kind="ExternalInput")
    b1_d = nc.dram_tensor("b1", (d_out, 1), F32, kind="ExternalInput")
    w2s_d = nc.dram_tensor("w2s", (d_out, d_out), BF16, kind="ExternalInput")
    upb_d = nc.dram_tensor("upb", (nchunk, 128, upc, d_out), BF16,
                           kind="ExternalInput")
    if GATHER_MODE == "indirect":
        offt_d = nc.dram_tensor("offt", (nchunk, 128, nj), I32,
                                kind="ExternalInput")
    else:
        ni = K * chunk
        offt_d = nc.dram_tensor("offt", (nchunk, 128, ni // 16), I16,
                                kind="ExternalInput")
    out_d = nc.dram_tensor("out", (nchunk, 128, upc, d_out), BF16,
                           kind="ExternalOutput")
    h2x_d = nc.dram_tensor("h2x", (ld, d_out), BF16, kind="Internal")

    relu = mybir.ActivationFunctionType.Relu

    # h2x rows are stored tile-major: the row holding down point
    # m = b*mlp_block + s*128 + p lives at m' = b*mlp_block + p*spb + s, so
    # each partition writes one contiguous spb*256B line per block store
    # (128 descriptors per store instead of 128*spb). The host remaps idx
    # values by the same permutation.
    h2x_blk = h2x_d.ap().rearrange("(b p s) f -> b p s f", b=nblk, p=128, s=spb)

    with tile.TileContext(nc) as tc, ExitStack() as ctx:
        consts = ctx.enter_context(tc.tile_pool(name="consts", bufs=1))
        w1_t = consts.tile([128, nkb, d_out], BF16)
        w1_v = w1_d.ap().rearrange("(j p) e -> j p e", j=nkb, p=128)
        for j in range(nkb):
            nc.sync.dma_start(w1_t[:, j, :], w1_v[j])
        w2s_t = consts.tile([128, d_out], BF16)
        nc.sync.dma_start(w2s_t[:], w2s_d.ap())
        b1_t = consts.tile([128, 1], F32)
        nc.sync.dma_start(b1_t[:], b1_d.ap())

        # ---------------- Phase A: MLP ----------------
        with tc.tile_pool(name="dn", bufs=3) as dn_pool, \
             tc.tile_pool(name="ps1", bufs=2, space="PSUM") as ps1_pool, \
             tc.tile_pool(name="hT", bufs=3) as hT_pool, \
             tc.tile_pool(name="ps2", bufs=4, space="PSUM") as ps2_pool, \
             tc.tile_pool(name="h2b", bufs=2) as h2b_pool:
            for b in range(nblk):
                dn = dn_pool.tile([128, nkb, mlp_block], BF16)
                for j in range(nkb):
                    nc.sync.dma_start(
                        dn[:, j, :],
                        downt_d.ap()[j * 128:(j + 1) * 128,
                                     b * mlp_block:(b + 1) * mlp_block])
                # ps1 = W1^T @ down_blk  -> [d_hidden, mlp_block]
                ps1 = ps1_pool.tile([128, mlp_block], F32)
                for j in range(nkb):
                    nc.tensor.matmul(ps1[:], w1_t[:, j, :], dn[:, j, :],
                                     start=(j == 0), stop=(j == nkb - 1))
                hT = hT_pool.tile([128, mlp_block], BF16)
                nc.scalar.activation(hT[:], ps1[:], relu, bias=b1_t[:])
                h2b = h2b_pool.tile([128, spb, d_out], BF16)
                for s in range(spb):
                    # ps2 = hT_s^T @ W2s -> [points(128), d_out] point-major
                    ps2 = ps2_pool.tile([128, d_out], F32)
                    nc.tensor.matmul(ps2[:], hT[:, s * 128:(s + 1) * 128],
                                     w2s_t[:], start=True, stop=True)
                    nc.vector.tensor_copy(h2b[:, s, :], ps2[:])
                nc.sync.dma_start(h2x_blk[b], h2b[:])

        # ---------------- Phase B: gather + combine ----------------
        with tc.tile_pool(name="gat", bufs=5) as g_pool, \
             tc.tile_pool(name="upt", bufs=3) as up_pool, \
             tc.tile_pool(name="t1", bufs=2) as t1_pool, \
             tc.tile_pool(name="ix", bufs=6) as ix_pool:
            ni = K * chunk
            for c in range(nchunk):
                g = g_pool.tile([128, nj, d_out], BF16)
                ix = ix_pool.tile([128, ni // 16], I16)
                nc.sync.dma_start(ix[:], offt_d.ap()[c])
                # host packs chunk 0 with points whose sources all lie in
                # h2x rows [0, ld/2) and chunk 1 within [0, 3*ld/4), so these
                # gathers depend only on a prefix of the MLP output and
                # overlap its tail
                pfx = {0: ld // 2, 1: 3 * ld // 4}.get(c, ld)
                nc.gpsimd.dma_gather(
                    g[:], h2x_d.ap()[:pfx, :], ix[:],
                    num_idxs=ni, num_idxs_reg=ni, elem_size=d_out,
                    single_packet=False, queue_num=c % 4)
                upt = up_pool.tile([128, upc, d_out], BF16)
                nc.sync.dma_start(upt[:], upb_d.ap()[c])
                gv = g[:].rearrange("p (u k) f -> p u k f", k=K)
                t1 = t1_pool.tile([128, upc, d_out], BF16)
                nc.vector.tensor_add(t1[:], gv[:, :, 0, :], gv[:, :, 1, :])
                nc.vector.tensor_add(t1[:], t1[:], gv[:, :, 2, :])
                nc.vector.tensor_add(t1[:], t1[:], upt[:])
                nc.sync.dma_start(out_d.ap()[c], t1[:])

    nc.compile()
    _BUILD_CACHE[key] = nc
    return nc


def _point_perm(idxn, lu, chunk):
    """Permutation of up-points: chunk 0 gets points with all sources in the
    first half of the table, chunk 1 in the first three quarters (sources are
    uniform, so candidates outnumber the 2*chunk needed ~8x)."""
    m0 = (idxn < LD // 2).all(axis=1)
    c0 = np.flatnonzero(m0)[:chunk]
    rest_mask = np.ones(lu, dtype=bool)
    rest_mask[c0] = False
    m1 = rest_mask & (idxn < 3 * LD // 4).all(axis=1)
    c1 = np.flatnonzero(m1)[:chunk]
    rest_mask[c1] = False
    pi = np.concatenate([c0, c1, np.flatnonzero(rest_mask)])
    assert len(c0) == chunk and len(c1) == chunk and pi.size == lu
    return pi


def _prep_core_inputs(down_features, up_features, idx, W1, b1, W2, b2, n,
                      ld=LD, lu=LU, d_in=D_IN, d_out=D_OUT, chunk=CHUNK):
    """Host-side packing of the full inputs into core n's input map.
    Returns (in_map, pi) where pi is the up-point permutation applied."""
    nchunk = lu // chunk
    upc = chunk // 128
    nj = K * upc

    downt = np.ascontiguousarray(down_features[:, n, :].T).astype(BF)
    blk = MLP_BLOCK
    spb = blk // 128
    m = np.arange(LD)
    b_, r_ = m // blk, m % blk
    sigma = (b_ * blk + (r_ % 128) * spb + r_ // 128)   # m -> h2x row m'

    idx_rm = sigma[idx[:, n, :]]                         # remapped sources (lu, K)
    pi = _point_perm(idx_rm, lu, chunk)
    idx_rm = idx_rm[pi]

    upb = up_features[pi, n, :].astype(np.float32) + b2[None, :].astype(np.float32)
    # (lu, d_out) -> [c, u, p, f] -> [c, p, u, f]
    upb = np.ascontiguousarray(
        upb.reshape(nchunk, upc, 128, d_out).transpose(0, 2, 1, 3)).astype(BF)

    if GATHER_MODE == "indirect":
        idxn = idx_rm.astype(np.int32)               # (lu, K)
        # point i = c*chunk + 128*u + p, neighbor k -> want[p, 3u+k];
        # indirect1d consumes iteration t from offs[t % 128, t // 128] and
        # writes dst chunk t = p*nj + j  ->  offs[t % 128, t // 128] = want
        want = np.ascontiguousarray(
            idxn.reshape(nchunk, upc, 128, K).transpose(0, 2, 1, 3)
        ).reshape(nchunk, 128 * nj)
        t = np.arange(128 * nj)
        offt = np.empty((nchunk, 128, nj), dtype=np.int32)
        offt[:, t % 128, t // 128] = want
    else:
        ni = K * chunk
        idxn = idx_rm.astype(np.int16)               # (lu, K), values < 2^15
        # gather slot i lands at partition i%128, 256B free slot i//128;
        # want point (c, p, u), neighbor k at free slot j=3u+k of partition p
        # -> slot i = (3u+k)*128 + p
        perm = idxn.reshape(nchunk, upc, 128, K).transpose(0, 1, 3, 2)  # [c,u,k,p]
        flat = perm.reshape(nchunk, ni)                                  # slot-major
        wrapped = flat.reshape(nchunk, ni // 16, 16).transpose(0, 2, 1)  # [c,16,ni/16]
        offt = np.ascontiguousarray(np.tile(wrapped, (1, 8, 1)))         # [c,128,ni/16]

    return {
        "downt": downt,
        "w1": np.ascontiguousarray(W1).astype(BF),
        "b1": np.ascontiguousarray(b1.astype(np.float32).reshape(d_out, 1)),
        "w2s": np.ascontiguousarray(W2.astype(np.float32) / np.float32(K)).astype(BF),
        "upb": upb,
        "offt": offt,
    }, pi


def _unpack_out(out_np, lu=LU, d_out=D_OUT, chunk=CHUNK):
    nchunk = lu // chunk
    upc = chunk // 128
    return np.ascontiguousarray(
        out_np.astype(np.float32).reshape(nchunk, 128, upc, d_out)
        .transpose(0, 2, 1, 3)
    ).reshape(lu, d_out)


def kernel(down_features, up_features, idx, W1, b1, W2, b2):
    down_features = np.asarray(down_features)
    up_features = np.asarray(up_features)
    idx = np.asarray(idx)
    W1, b1, W2, b2 = (np.asarray(a) for a in (W1, b1, W2, b2))

    nc = _build()
    prepped = [
        _prep_core_inputs(down_features, up_features, idx, W1, b1, W2, b2, n)
        for n in range(NCORES)
    ]
    in_maps = [p[0] for p in prepped]
    res = run_bass_kernel_spmd(nc, in_maps, core_ids=list(range(NCORES)))
    cols = []
    for n in range(NCORES):
        permuted = _unpack_out(res.results[n]["out"])
        col = np.empty_like(permuted)
        col[prepped[n][1]] = permuted
        cols.append(col)
    return np.stack(cols, axis=1).astype(np.float32)


# revision 16
# speedup vs baseline: 1.1384x; 1.1384x over previous
"""Trainium2 Bass kernel for KNN-upsample (MLP on down points + KNN mean + residual).

Contract: kernel(**inputs) takes FULL numpy inputs (as produced by
setup_inputs) and returns the FULL (LU, N, D_OUT) float32 output.

Sharding: batch axis N=8 -> one NeuronCore per batch column (data
parallel, no cross-core communication).

Per-core device program (all bf16 on the wires, fp32 PSUM accumulate):
  Phase A: h2x[m, :] = relu(down[m, :] @ W1 + b1) @ (W2/3)   -> DRAM scratch (bf16)
  Phase B: out[i, :] = (up[i, :] + b2) + sum_k h2x[idx[i, k], :]
           gather via gpsimd.dma_gather (bf16 256B rows), rotated across the
           4 SWDGE queues so descriptor generation for consecutive chunks
           runs on different Q7 core pairs and pipelines; deep tile-pool
           buffering (g=5/ix=6) keeps that pipeline full.
           K-sum + residual on DVE in bf16 (2x mode, contiguous rows).

Host-side preprocessing (pure data-layout work):
  - down column transposed to (D_IN, LD) bf16 so matmul lhsT/rhs layouts are native
  - W2 pre-scaled by 1/3 (folds the k-mean), b2 folded into up; weights bf16
  - idx converted to int32 offset tables offt[c, p, 3u+k] = idx[point(c,p,u), k]
    where point i = c*CHUNK + 128*u + p, so gathered rows land at g[p, 3u+k, :]
  - up / out stored chunk-major [c, p, u, f] bf16 so DMAs are contiguous per
    partition; the inverse permutation (and fp32 cast) applied on host
"""

import numpy as np
import ml_dtypes
from contextlib import ExitStack

import concourse.bacc as bacc
import concourse.bass as bass
import concourse.tile as tile
import concourse.mybir as mybir
from concourse.bass_utils import run_bass_kernel_spmd

LD, LU, N, D_IN, D_OUT, K = 16384, 65536, 8, 256, 128, 3

CHUNK = 4096            # upsample points per gather chunk
MLP_BLOCK = 512         # down points per MLP compute block
NCORES = 8
GATHER_MODE = "dma_gather"   # "dma_gather" | "indirect"

F32 = mybir.dt.float32
BF16 = mybir.dt.bfloat16
I32 = mybir.dt.int32
I16 = mybir.dt.int16
BF = ml_dtypes.bfloat16

_BUILD_CACHE = {}


def _build(ld=LD, lu=LU, d_in=D_IN, d_out=D_OUT, chunk=CHUNK, mlp_block=MLP_BLOCK):
    """Build + compile the per-core Bass program (identical on all cores)."""
    key = (ld, lu, d_in, d_out, chunk, mlp_block)
    if key in _BUILD_CACHE:
        return _BUILD_CACHE[key]

    nchunk = lu // chunk
    upc = chunk // 128                  # 128-point groups per chunk
    nj = K * upc                        # gather slots per partition per chunk
    nkb = d_in // 128                   # K-tiles for matmul 1
    nblk = ld // mlp_block              # MLP compute blocks
    spb = mlp_block // 128              # 128-point sub-tiles per MLP block

    # 4 SWDGE queues: consecutive gathers rotate queues so each has its own
    # descriptor ring + completion semaphore; otherwise gather c+1 stalls on
    # gather c's DMA drain (~16us bubble per chunk).
    nc = bacc.Bacc("TRN2", target_bir_lowering=False, debug=False,
                   num_swdge_queues=4)

    downt_d = nc.dram_tensor("downt", (d_in, ld), BF16, kind="ExternalInput")
    w1_d = nc.dram_tensor("w1", (d_in, d_out), BF16, kind="ExternalInput")
    b1_d = nc.dram_tensor("b1", (d_out, 1), F32, kind="ExternalInput")
    w2s_d = nc.dram_tensor("w2s", (d_out, d_out), BF16, kind="ExternalInput")
    upb_d = nc.dram_tensor("upb", (nchunk, 128, upc, d_out), BF16,
                           kind="ExternalInput")
    if GATHER_MODE == "indirect":
        offt_d = nc.dram_tensor("offt", (nchunk, 128, nj), I32,
                                kind="ExternalInput")
    else:
        ni = K * chunk
        offt_d = nc.dram_tensor("offt", (nchunk, 128, ni // 16), I16,
                                kind="ExternalInput")
    out_d = nc.dram_tensor("out", (nchunk, 128, upc, d_out), BF16,
                           kind="ExternalOutput")
    h2x_d = nc.dram_tensor("h2x", (ld, d_out), BF16, kind="Internal")

    relu = mybir.ActivationFunctionType.Relu

    # h2x rows are stored tile-major: the row holding down point
    # m = b*mlp_block + s*128 + p lives at m' = b*mlp_block + p*spb + s, so
    # each partition writes one contiguous spb*256B line per block store
    # (128 descriptors per store instead of 128*spb). The host remaps idx
    # values by the same permutation.
    h2x_blk = h2x_d.ap().rearrange("(b p s) f -> b p s f", b=nblk, p=128, s=spb)

    with tile.TileContext(nc) as tc, ExitStack() as ctx:
        consts = ctx.enter_context(tc.tile_pool(name="consts", bufs=1))
        w1_t = consts.tile([128, nkb, d_out], BF16)
        w1_v = w1_d.ap().rearrange("(j p) e -> j p e", j=nkb, p=128)
        for j in range(nkb):
            nc.sync.dma_start(w1_t[:, j, :], w1_v[j])
        w2s_t = consts.tile([128, d_out], BF16)
        nc.sync.dma_start(w2s_t[:], w2s_d.ap())
        b1_t = consts.tile([128, 1], F32)
        nc.sync.dma_start(b1_t[:], b1_d.ap())

        # ---------------- Phase A: MLP ----------------
        with tc.tile_pool(name="dn", bufs=3) as dn_pool, \
             tc.tile_pool(name="ps1", bufs=2, space="PSUM") as ps1_pool, \
             tc.tile_pool(name="hT", bufs=3) as hT_pool, \
             tc.tile_pool(name="ps2", bufs=4, space="PSUM") as ps2_pool, \
             tc.tile_pool(name="h2b", bufs=2) as h2b_pool:
            for b in range(nblk):
                dn = dn_pool.tile([128, nkb, mlp_block], BF16)
                for j in range(nkb):
                    nc.sync.dma_start(
                        dn[:, j, :],
                        downt_d.ap()[j * 128:(j + 1) * 128,
                                     b * mlp_block:(b + 1) * mlp_block])
                # ps1 = W1^T @ down_blk  -> [d_hidden, mlp_block]
                ps1 = ps1_pool.tile([128, mlp_block], F32)
                for j in range(nkb):
                    nc.tensor.matmul(ps1[:], w1_t[:, j, :], dn[:, j, :],
                                     start=(j == 0), stop=(j == nkb - 1))
                hT = hT_pool.tile([128, mlp_block], BF16)
                nc.scalar.activation(hT[:], ps1[:], relu, bias=b1_t[:])
                h2b = h2b_pool.tile([128, spb, d_out], BF16)
                for s in range(spb):
                    # ps2 = hT_s^T @ W2s -> [points(128), d_out] point-major
                    ps2 = ps2_pool.tile([128, d_out], F32)
                    nc.tensor.matmul(ps2[:], hT[:, s * 128:(s + 1) * 128],
                                     w2s_t[:], start=True, stop=True)
                    nc.vector.tensor_copy(h2b[:, s, :], ps2[:])
                nc.sync.dma_start(h2x_blk[b], h2b[:])

        # ---------------- Phase B: gather + combine ----------------
        with tc.tile_pool(name="gat", bufs=5) as g_pool, \
             tc.tile_pool(name="upt", bufs=3) as up_pool, \
             tc.tile_pool(name="t1", bufs=2) as t1_pool, \
             tc.tile_pool(name="ix", bufs=6) as ix_pool:
            ni = K * chunk
            for c in range(nchunk):
                g = g_pool.tile([128, nj, d_out], BF16)
                ix = ix_pool.tile([128, ni // 16], I16)
                nc.sync.dma_start(ix[:], offt_d.ap()[c])
                nc.gpsimd.dma_gather(
                    g[:], h2x_d.ap(), ix[:],
                    num_idxs=ni, num_idxs_reg=ni, elem_size=d_out,
                    single_packet=False, queue_num=c % 4)
                upt = up_pool.tile([128, upc, d_out], BF16)
                nc.sync.dma_start(upt[:], upb_d.ap()[c])
                gv = g[:].rearrange("p (u k) f -> p u k f", k=K)
                t1 = t1_pool.tile([128, upc, d_out], BF16)
                nc.vector.tensor_add(t1[:], gv[:, :, 0, :], gv[:, :, 1, :])
                nc.vector.tensor_add(t1[:], t1[:], gv[:, :, 2, :])
                nc.vector.tensor_add(t1[:], t1[:], upt[:])
                nc.sync.dma_start(out_d.ap()[c], t1[:])

    nc.compile()
    _BUILD_CACHE[key] = nc
    return nc


def _prep_core_inputs(down_features, up_features, idx, W1, b1, W2, b2, n,
                      ld=LD, lu=LU, d_in=D_IN, d_out=D_OUT, chunk=CHUNK):
    """Host-side packing of the full inputs into core n's input map."""
    nchunk = lu // chunk
    upc = chunk // 128
    nj = K * upc

    downt = np.ascontiguousarray(down_features[:, n, :].T).astype(BF)
    upb = up_features[:, n, :].astype(np.float32) + b2[None, :].astype(np.float32)
    # (lu, d_out) -> [c, u, p, f] -> [c, p, u, f]
    upb = np.ascontiguousarray(
        upb.reshape(nchunk, upc, 128, d_out).transpose(0, 2, 1, 3)).astype(BF)

    blk = MLP_BLOCK
    spb = blk // 128
    m = np.arange(LD)
    b_, r_ = m // blk, m % blk
    sigma = (b_ * blk + (r_ % 128) * spb + r_ // 128)   # m -> h2x row m'

    if GATHER_MODE == "indirect":
        idxn = sigma[idx[:, n, :]].astype(np.int32)  # (lu, K)
        # point i = c*chunk + 128*u + p, neighbor k -> want[p, 3u+k];
        # indirect1d consumes iteration t from offs[t % 128, t // 128] and
        # writes dst chunk t = p*nj + j  ->  offs[t % 128, t // 128] = want
        want = np.ascontiguousarray(
            idxn.reshape(nchunk, upc, 128, K).transpose(0, 2, 1, 3)
        ).reshape(nchunk, 128 * nj)
        t = np.arange(128 * nj)
        offt = np.empty((nchunk, 128, nj), dtype=np.int32)
        offt[:, t % 128, t // 128] = want
    else:
        ni = K * chunk
        idxn = sigma[idx[:, n, :]].astype(np.int16)  # (lu, K), values < 2^15
        # gather slot i lands at partition i%128, 256B free slot i//128;
        # want point (c, p, u), neighbor k at free slot j=3u+k of partition p
        # -> slot i = (3u+k)*128 + p
        perm = idxn.reshape(nchunk, upc, 128, K).transpose(0, 1, 3, 2)  # [c,u,k,p]
        flat = perm.reshape(nchunk, ni)                                  # slot-major
        wrapped = flat.reshape(nchunk, ni // 16, 16).transpose(0, 2, 1)  # [c,16,ni/16]
        offt = np.ascontiguousarray(np.tile(wrapped, (1, 8, 1)))         # [c,128,ni/16]

    return {
        "downt": downt,
        "w1": np.ascontiguousarray(W1).astype(BF),
        "b1": np.ascontiguousarray(b1.astype(np.float32).reshape(d_out, 1)),
        "w2s": np.ascontiguousarray(W2.astype(np.float32) / np.float32(K)).astype(BF),
        "upb": upb,
        "offt": offt,
    }


def _unpack_out(out_np, lu=LU, d_out=D_OUT, chunk=CHUNK):
    nchunk = lu // chunk
    upc = chunk // 128
    return np.ascontiguousarray(
        out_np.astype(np.float32).reshape(nchunk, 128, upc, d_out)
        .transpose(0, 2, 1, 3)
    ).reshape(lu, d_out)


def kernel(down_features, up_features, idx, W1, b1, W2, b2):
    down_features = np.asarray(down_features)
    up_features = np.asarray(up_features)
    idx = np.asarray(idx)
    W1, b1, W2, b2 = (np.asarray(a) for a in (W1, b1, W2, b2))

    nc = _build()
    in_maps = [
        _prep_core_inputs(down_features, up_features, idx, W1, b1, W2, b2, n)
        for n in range(NCORES)
    ]
    res = run_bass_kernel_spmd(nc, in_maps, core_ids=list(range(NCORES)))
    cols = [_unpack_out(res.results[n]["out"]) for n in range(NCORES)]
    return np.stack(cols, axis=1).astype(np.float32)
